# revision 32
# baseline (speedup 1.0000x reference)
"""GQA attention (B=2, S=2048, H=4096, 32 Q heads / 8 KV heads, RoPE, non-causal)
distributed over 8 trn2 NeuronCores.

Sharding: 8-way tensor parallel over heads. Core c owns Q heads 4c..4c+3 and
KV head c (GQA group maps exactly: q head h uses kv head h//4). Each core
computes QKV projections for ALL B*S=4096 tokens on its head shard, full
attention for its 4 heads x 2 batches, then AllToAll re-shards from
head-parallel to token-parallel and each core runs the output projection for
its 512 tokens against the full w_out.

Device compute in bf16 (f32 PSUM accumulation; softmax exp/sum in f32, no
max-subtraction: logits are O(5) here so exp is safe). Projections produce
qT/kT = [head_dim, tokens]; scoresT[k,q] tiles come from lhsT=kT_tile,
rhs=qT. Softmax denominators come from an all-ones [128,128] stationary
matmul (row sums broadcast across partitions for free); normalization happens
on PV-psum eviction. RoPE's rotate-half runs as an SBUF->SBUF partition-swap
DMA plus 3 DVE ops against host-built [cos;cos] and [-sin;sin] tables.

The A2A is split into three rounds (heads 0-1 after head 1's attention,
head 2, head 3) so communication overlaps attention; the output projection
is emitted software-pipelined (two PSUM groups alive) with the round-3-
dependent k-tiles trailing, so the final A2A flight also hides under matmuls.
"""
import sys, types
import numpy as np
import ml_dtypes

HIDDEN = 4096
HEAD_DIM = 128
N_HEADS = 32
N_KV_HEADS = 8
ROPE_THETA = 10000.0
B, S = 2, 2048
NC = 8
HPC = N_HEADS // NC          # 4 q heads / core
T = B * S                    # 4096 flat tokens
TCH = T // NC                # 512 tokens / core post-A2A
P = 128
QK_D = HPC * HEAD_DIM + HEAD_DIM   # 640 projected dims per core (4 q + 1 k)
NDQ = QK_D // P              # 5 dq tiles (0-3 Q heads, 4 K head)
HT = HIDDEN // P             # 32 hidden tiles
TOKC = 512                   # stage-1 token chunk
NTC = T // TOKC              # 8 chunks
KT = S // P                  # 16 k tiles per batch
QC = S // 512                # 4 q chunks of 512 per batch


def _install_ntff_hook():
    try:
        import antenv.axon_hooks  # noqa
        return
    except ImportError:
        pass
    try:
        from trn_agent_boot.trn_boot import _ntff_profile_via_ctypes
        mod = types.ModuleType('antenv.axon_hooks')
        _hook = _ntff_profile_via_ctypes('/opt/axon/libaxon_pjrt.so')
        mod.get_axon_ntff_profile_hook = lambda: _hook
        mod.set_axon_ntff_profile_hook = lambda h: None
        sys.modules['antenv.axon_hooks'] = mod
    except Exception:
        pass


def build_nc():
    from contextlib import ExitStack
    import concourse.mybir as mybir
    import concourse.tile as tile
    from concourse import bacc
    from concourse.masks import make_identity

    dt = mybir.dt
    Alu = mybir.AluOpType
    Act = mybir.ActivationFunctionType

    nc = bacc.Bacc("TRN2", target_bir_lowering=False, num_devices=NC)
    xT = nc.declare_dram_parameter("xT", [HIDDEN, T], dt.bfloat16, isOutput=False)
    wqk = nc.declare_dram_parameter("wqk", [HIDDEN, QK_D], dt.bfloat16, isOutput=False)
    wv = nc.declare_dram_parameter("wv", [HIDDEN, HEAD_DIM], dt.bfloat16, isOutput=False)
    wout = nc.declare_dram_parameter("wout", [HIDDEN, HIDDEN], dt.bfloat16, isOutput=False)
    cos2 = nc.declare_dram_parameter("cos2", [P, T], dt.bfloat16, isOutput=False)
    sinS = nc.declare_dram_parameter("sinS", [P, T], dt.bfloat16, isOutput=False)
    out = nc.declare_dram_parameter("out", [TCH, HIDDEN], dt.float32, isOutput=True)

    rg = [list(range(NC))]

    with tile.TileContext(nc) as tc:
        with tc.tile_pool(name="persist", bufs=1) as persist, \
             tc.tile_pool(name="dram", bufs=1, space="DRAM") as dram:
            qkT_sb = persist.tile([P, NDQ * T], dt.bfloat16, tag="qkT")
            v_sb = persist.tile([P, HT * P], dt.bfloat16, tag="v")
            ones_sb = persist.tile([P, P], dt.bfloat16, tag="ones")
            ident_sb = persist.tile([P, P], dt.bfloat16, tag="ident")
            nc.vector.memset(ones_sb[:], 1.0)
            make_identity(nc, ident_sb[:])

            # Three A2A rounds: round 0 carries heads 0-1 rows of every dest
            # block, round 1 head 2, round 2 head 3.
            a2a_in1 = dram.tile([NC * 2 * P, TCH], dt.bfloat16, tag="a2a_i1")
            a2a_in2 = dram.tile([NC * P, TCH], dt.bfloat16, tag="a2a_i2")
            a2a_in3 = dram.tile([NC * P, TCH], dt.bfloat16, tag="a2a_i3")
            a2a_out1 = dram.tile([NC * 2 * P, TCH], dt.bfloat16, tag="a2a_o1")
            a2a_out2 = dram.tile([NC * P, TCH], dt.bfloat16, tag="a2a_o2")
            a2a_out3 = dram.tile([NC * P, TCH], dt.bfloat16, tag="a2a_o3")

            # ================= stage 1: projections + RoPE =================
            with tc.tile_pool(name="s1", bufs=1) as s1, \
                 tc.tile_pool(name="xc", bufs=2) as xcp, \
                 tc.tile_pool(name="rope_tmp", bufs=3) as rtp, \
                 tc.tile_pool(name="vstage", bufs=2) as vsp, \
                 tc.tile_pool(name="qkps", bufs=2, space="PSUM") as qkps, \
                 tc.tile_pool(name="vps", bufs=2, space="PSUM") as vps:
                # small per-DMA sizes spread the critical startup bytes over
                # many DMA queues (one dma_start rides one ~31GB/s queue)
                WG = 2                       # h-tiles per wqk sub-tile/DMA
                XG = 4                       # h-tiles per xc sub-tile/DMA
                wqk_sbs = [s1.tile([P, WG * QK_D], dt.bfloat16, tag=f"wqk{g}",
                                   name=f"wqk{g}")
                           for g in range(HT // WG)]
                xcs0 = [xcp.tile([P, XG * TOKC], dt.bfloat16, tag=f"xc{g}",
                                 name=f"xc0_{g}")
                        for g in range(HT // XG)]
                # interleave the first xc chunk's DMAs with the weight DMAs;
                # the first h-tiles go as single-tile DMAs so the very first
                # matmuls start as early as possible
                for h in range(4):
                    nc.sync.dma_start(
                        xcs0[0][:].rearrange("p (n m) -> p n m", n=XG)[:, h, :],
                        xT[h * P:(h + 1) * P, 0:TOKC])
                    nc.sync.dma_start(
                        wqk_sbs[h // WG][:].rearrange("p (n m) -> p n m", n=WG)[:, h % WG, :],
                        wqk[h * P:(h + 1) * P, :])
                for g in range(HT // XG):
                    if g > 0:
                        nc.sync.dma_start(
                            xcs0[g][:].rearrange("p (n m) -> p n m", n=XG),
                            xT[g * XG * P:(g + 1) * XG * P, 0:TOKC]
                            .rearrange("(n p) m -> p n m", p=P))
                    for gg in range(2):
                        w_g = g * 2 + gg
                        if w_g >= 2:
                            nc.sync.dma_start(
                                wqk_sbs[w_g][:].rearrange("p (n m) -> p n m", n=WG),
                                wqk[w_g * WG * P:(w_g + 1) * WG * P, :]
                                .rearrange("(n p) m -> p n m", p=P))
                wv_sb = s1.tile([P, HT * HEAD_DIM], dt.bfloat16, tag="wv")
                cos_sb = s1.tile([P, T], dt.bfloat16, tag="cos")
                sin_sb = s1.tile([P, T], dt.bfloat16, tag="sin")
                for g in range(4):
                    nc.sync.dma_start(
                        wv_sb[:].rearrange("p (n m) -> p n m", n=HT)[:, g * 8:(g + 1) * 8, :],
                        wv[g * 8 * P:(g + 1) * 8 * P, :].rearrange("(n p) m -> p n m", p=P))
                    nc.sync.dma_start(cos_sb[:, g * 1024:(g + 1) * 1024],
                                      cos2[:, g * 1024:(g + 1) * 1024])
                    nc.sync.dma_start(sin_sb[:, g * 1024:(g + 1) * 1024],
                                      sinS[:, g * 1024:(g + 1) * 1024])

                def wqk_tile(h, lo, hi):
                    return wqk_sbs[h // WG][:, (h % WG) * QK_D + lo:(h % WG) * QK_D + hi]

                grps = [list(range(0, 3)), list(range(3, NDQ))]
                for tci in range(NTC):
                    t0 = tci * TOKC
                    if tci == 0:
                        xcs = xcs0
                    else:
                        xcs = [xcp.tile([P, XG * TOKC], dt.bfloat16, tag=f"xc{g}",
                                        name=f"xc{tci}_{g}")
                               for g in range(HT // XG)]
                        for g, t_ in enumerate(xcs):
                            nc.sync.dma_start(
                                t_[:].rearrange("p (n m) -> p n m", n=XG),
                                xT[g * XG * P:(g + 1) * XG * P, t0:t0 + TOKC]
                                .rearrange("(n p) m -> p n m", p=P))

                    def xc_tile(h):
                        return xcs[h // XG][:, (h % XG) * TOKC:(h % XG + 1) * TOKC]

                    # ---- Q/K projection, psum [128, <=3*512] groups
                    for grp in grps:
                        ps = qkps.tile([P, len(grp) * TOKC], dt.float32, tag="qk",
                                       name=f"qk{tci}_{grp[0]}")
                        for h in range(HT):
                            for gi, dq in enumerate(grp):
                                nc.tensor.matmul(
                                    ps[:, gi * TOKC:(gi + 1) * TOKC],
                                    wqk_tile(h, dq * P, (dq + 1) * P),
                                    xc_tile(h),
                                    start=(h == 0), stop=(h == HT - 1))
                        # ---- RoPE on each dq tile -> qkT_sb (bf16)
                        for gi, dq in enumerate(grp):
                            qkf = rtp.tile([P, TOKC], dt.float32, tag="qkf")
                            nc.scalar.copy(qkf[:], ps[:, gi * TOKC:(gi + 1) * TOKC])
                            qksw = rtp.tile([P, TOKC], dt.float32, tag="qksw")
                            nc.sync.dma_start(qksw[0:64, :], qkf[64:128, :])
                            nc.sync.dma_start(qksw[64:128, :], qkf[0:64, :])
                            tm1 = rtp.tile([P, TOKC], dt.float32, tag="tm1")
                            tm2 = rtp.tile([P, TOKC], dt.float32, tag="tm2")
                            nc.vector.tensor_tensor(
                                tm1[:], qkf[:], cos_sb[:, t0:t0 + TOKC], Alu.mult)
                            nc.vector.tensor_tensor(
                                tm2[:], qksw[:], sin_sb[:, t0:t0 + TOKC], Alu.mult)
                            nc.vector.tensor_tensor(
                                qkT_sb[:, dq * T + t0: dq * T + t0 + TOKC],
                                tm1[:], tm2[:], Alu.add)

                    # ---- V projection (vT orientation) + transpose to v_sb
                    vtp = vps.tile([P, TOKC], dt.float32, tag="vt",
                                   name=f"vt{tci}")
                    for h in range(HT):
                        nc.tensor.matmul(
                            vtp[:], wv_sb[:, h * HEAD_DIM:(h + 1) * HEAD_DIM],
                            xc_tile(h),
                            start=(h == 0), stop=(h == HT - 1))
                    vt_sb = vsp.tile([P, TOKC], dt.bfloat16, tag="vt_sb")
                    nc.scalar.copy(vt_sb[:], vtp[:])
                    for j in range(TOKC // P):
                        tp = vps.tile([P, P], dt.bfloat16, tag="vt",
                                      name=f"vtr{tci}_{j}")
                        nc.tensor.transpose(tp[:], vt_sb[:, j * P:(j + 1) * P], ident_sb[:])
                        ti = (t0 + j * P) // P
                        nc.vector.tensor_copy(v_sb[:, ti * P:(ti + 1) * P], tp[:])

            # ======== stages 2+3 share the 'late' pools (attnT + wout) ========
            with tc.tile_pool(name="late", bufs=1) as late, \
                 tc.tile_pool(name="wo", bufs=24) as wop:
                attnT_sb = late.tile([P, HT * TCH], dt.bfloat16, tag="attnT")
                kts_a = [kt for kt in range(HT) if kt % 4 != 3]   # A2A rounds 1+2
                kts_b = [kt for kt in range(HT) if kt % 4 == 3]   # A2A round 3
                pref_wo = {}

                # attnT row tile kt: block i=kt//4, r=kt%4. r in {0,1} delivered
                # by A2A round 1, r==2 by round 2, r==3 by round 3. Loads issue
                # on the gpsimd queue right after the producing collective.
                def load_attnT(round_):
                    if round_ == 0:
                        for i in range(NC):
                            for r in range(2):
                                nc.gpsimd.dma_start(
                                    attnT_sb[:, (4 * i + r) * TCH:(4 * i + r + 1) * TCH],
                                    a2a_out1[(2 * i + r) * P:(2 * i + r + 1) * P, :])
                    else:
                        buf = a2a_out2 if round_ == 1 else a2a_out3
                        r = round_ + 1
                        for i in range(NC):
                            nc.gpsimd.dma_start(
                                attnT_sb[:, (4 * i + r) * TCH:(4 * i + r + 1) * TCH],
                                buf[i * P:(i + 1) * P, :])

                def prefetch_wo(n):
                    for kt in kts_a:
                        wo = wop.tile([P, 512], dt.bfloat16, tag="wo",
                                      name=f"wo{n}_{kt}")
                        nc.sync.dma_start(
                            wo[:], wout[kt * P:(kt + 1) * P, n * 512:(n + 1) * 512])
                        pref_wo[(n, kt)] = wo

                # ================= stage 2: attention =================
                with ExitStack() as st2:
                    pbp = st2.enter_context(tc.tile_pool(name="probsT", bufs=16))
                    aev = st2.enter_context(tc.tile_pool(name="attn_ev", bufs=3))
                    rcp = st2.enter_context(tc.tile_pool(name="recip", bufs=2))
                    sps = st2.enter_context(tc.tile_pool(name="sps", bufs=2, space="PSUM"))
                    pvps = st2.enter_context(tc.tile_pool(name="pvps", bufs=2, space="PSUM"))
                    smps = st2.enter_context(tc.tile_pool(name="smps", bufs=2, space="PSUM"))
                    for h in range(HPC):
                        for b in range(B):
                            boff = b * S
                            # ---- probsT = exp(kT.T @ qT) tiles [128k, 2048q];
                            # tree level-1 pair sums interleave with the exps
                            probs = []
                            stack = []
                            for kt in range(KT):
                                pb = pbp.tile([P, S], dt.bfloat16, tag="pb",
                                              name=f"pb{h}_{b}_{kt}")
                                probs.append(pb)
                                for half in range(2):
                                    sc = sps.tile([P, 2 * 512], dt.float32, tag="sc",
                                                  name=f"sc{h}_{b}_{kt}_{half}")
                                    for qi in range(2):
                                        q0 = boff + half * 1024 + qi * 512
                                        nc.tensor.matmul(
                                            sc[:, qi * 512:(qi + 1) * 512],
                                            qkT_sb[:, HPC * T + boff + kt * P:
                                                   HPC * T + boff + (kt + 1) * P],
                                            qkT_sb[:, h * T + q0: h * T + q0 + 512],
                                            start=True, stop=True)
                                    nc.scalar.activation(
                                        pb[:, half * 1024:(half + 1) * 1024], sc[:], Act.Exp)
                                # binary-counter pair reduction: merge equal-
                                # level partial sums as soon as both exist
                                stack.append((probs[kt], 0))
                                while len(stack) >= 2 and stack[-1][1] == stack[-2][1]:
                                    (ta, la), (tb, _) = stack[-2], stack[-1]
                                    stack = stack[:-2]
                                    t_ = pbp.tile([P, S], dt.bfloat16, tag="pbs", bufs=6,
                                                  name=f"ps{h}_{b}_{kt}_{la}")
                                    eng = nc.gpsimd if la == 0 else nc.vector
                                    eng.tensor_tensor(t_[:], ta[:], tb[:], Alu.add)
                                    stack.append((t_, la + 1))
                            psum_tile = stack[0][0]
                            # ---- PV per 512-q chunk, evicted unnormalized
                            aus = []
                            for qc in range(QC):
                                qs = slice(qc * 512, (qc + 1) * 512)
                                pv = pvps.tile([P, 512], dt.float32, tag="pv",
                                               name=f"pv{h}_{b}_{qc}")
                                for kt in range(KT):
                                    nc.tensor.matmul(
                                        pv[:], v_sb[:, (b * KT + kt) * P:(b * KT + kt + 1) * P],
                                        probs[kt][:, qs],
                                        start=(kt == 0), stop=(kt == KT - 1))
                                au = aev.tile([P, 512], dt.bfloat16, tag="au", bufs=6,
                                              name=f"au{h}_{b}_{qc}")
                                nc.scalar.copy(au[:], pv[:])
                                aus.append(au)
                            # ---- broadcast row sums + normalize + ship
                            for qc in range(QC):
                                qs = slice(qc * 512, (qc + 1) * 512)
                                sm = smps.tile([P, 512], dt.float32, tag="sm",
                                               name=f"sm{h}_{b}_{qc}")
                                nc.tensor.matmul(
                                    sm[:], ones_sb[:], psum_tile[:, qs],
                                    start=True, stop=True)
                                rc = rcp.tile([P, 512], dt.float32, tag="rc")
                                nc.vector.reciprocal_approx_fast(rc[:], sm[:])
                                at = aev.tile([P, 512], dt.bfloat16, tag="at")
                                nc.vector.tensor_tensor(at[:], aus[qc][:], rc[:], Alu.mult)
                                j = b * QC + qc
                                if h < 2:
                                    nc.sync.dma_start(
                                        a2a_in1[j * 256 + h * P: j * 256 + (h + 1) * P, :],
                                        at[:])
                                else:
                                    buf = a2a_in2 if h == 2 else a2a_in3
                                    nc.sync.dma_start(buf[j * P:(j + 1) * P, :], at[:])
                        if h == 1:
                            nc.gpsimd.collective_compute(
                                "AllToAll", Alu.bypass, replica_groups=rg,
                                ins=[a2a_in1[:].opt()], outs=[a2a_out1[:].opt()])
                        elif h == 2:
                            nc.gpsimd.collective_compute(
                                "AllToAll", Alu.bypass, replica_groups=rg,
                                ins=[a2a_in2[:].opt()], outs=[a2a_out2[:].opt()])
                            prefetch_wo(0)
                    nc.gpsimd.collective_compute(
                        "AllToAll", Alu.bypass, replica_groups=rg,
                        ins=[a2a_in3[:].opt()], outs=[a2a_out3[:].opt()])
                    # all attnT loads AFTER the last doorbell: the gpsimd
                    # stream never blocks a later collective's trigger on an
                    # earlier collective's completion
                    load_attnT(0)
                    load_attnT(1)
                    load_attnT(2)

                # ================= stage 3: output projection =================
                with tc.tile_pool(name="oev", bufs=4) as oev, \
                     tc.tile_pool(name="ops", bufs=8, space="PSUM") as ops:
                    NTT = TCH // P
                    NN = HIDDEN // 512
                    psls = {}

                    def get_wo(n, kt):
                        if (n, kt) in pref_wo:
                            return pref_wo.pop((n, kt))
                        wo = wop.tile([P, 512], dt.bfloat16, tag="wo",
                                      name=f"wo{n}_{kt}")
                        nc.sync.dma_start(
                            wo[:], wout[kt * P:(kt + 1) * P, n * 512:(n + 1) * 512])
                        return wo

                    def part_a(n):
                        psls[n] = [ops.tile([P, 512], dt.float32, tag="o",
                                            name=f"o{n}_{i}") for i in range(NTT)]
                        for ki, kt in enumerate(kts_a):
                            wo = get_wo(n, kt)
                            for tt in range(NTT):
                                nc.tensor.matmul(
                                    psls[n][tt][:],
                                    attnT_sb[:, kt * TCH + tt * P: kt * TCH + (tt + 1) * P],
                                    wo[:], start=(ki == 0), stop=False)

                    def part_b(n):
                        for ki, kt in enumerate(kts_b):
                            wo = get_wo(n, kt)
                            for tt in range(NTT):
                                nc.tensor.matmul(
                                    psls[n][tt][:],
                                    attnT_sb[:, kt * TCH + tt * P: kt * TCH + (tt + 1) * P],
                                    wo[:], start=False, stop=(ki == len(kts_b) - 1))
                        for tt in range(NTT):
                            ob = oev.tile([P, 512], dt.float32, tag="ob")
                            nc.scalar.copy(ob[:], psls[n][tt][:])
                            nc.sync.dma_start(
                                out[tt * P:(tt + 1) * P, n * 512:(n + 1) * 512], ob[:])

                    # software-pipelined emission: two psum groups alive; the
                    # round-3-dependent MMs of group n trail group n+1's part_a
                    part_a(0)
                    for n in range(1, NN):
                        part_a(n)
                        part_b(n - 1)
                    part_b(NN - 1)
    nc.compile()
    return nc


_NC_CACHE = None


def _rope_tables():
    half = HEAD_DIM // 2
    inv_freq = 1.0 / (ROPE_THETA ** (np.arange(half, dtype=np.float64) / half))
    freqs = np.arange(S, dtype=np.float64)[:, None] * inv_freq[None, :]
    cos = np.tile(np.cos(freqs), (B, 1)).T   # [64, T]
    sin = np.tile(np.sin(freqs), (B, 1)).T
    bf = ml_dtypes.bfloat16
    cos2 = np.concatenate([cos, cos], axis=0).astype(bf)      # [128, T]
    sinS = np.concatenate([-sin, sin], axis=0).astype(bf)     # [128, T]
    return cos2, sinS


def kernel(x, w_qkv, w_out):
    return _run(x, w_qkv, w_out, trace=False)[0]


def _run(x, w_qkv, w_out, trace=False):
    _install_ntff_hook()
    from concourse.bass_utils import run_bass_kernel_spmd

    global _NC_CACHE
    if _NC_CACHE is None:
        _NC_CACHE = build_nc()
    nc = _NC_CACHE

    x = np.asarray(x, dtype=np.float32)
    w_qkv = np.asarray(w_qkv, dtype=np.float32)
    w_out = np.asarray(w_out, dtype=np.float32)

    bf = ml_dtypes.bfloat16
    scale = HEAD_DIM ** -0.5
    xT_h = np.ascontiguousarray(x.reshape(T, HIDDEN).T).astype(bf)
    wout_h = w_out.astype(bf)
    cos_h, sin_h = _rope_tables()

    in_maps = []
    for c in range(NC):
        wq = w_qkv[:, c * HPC * HEAD_DIM:(c + 1) * HPC * HEAD_DIM] * scale
        k0 = N_HEADS * HEAD_DIM + c * HEAD_DIM
        v0 = (N_HEADS + N_KV_HEADS) * HEAD_DIM + c * HEAD_DIM
        wqk_h = np.concatenate([wq, w_qkv[:, k0:k0 + HEAD_DIM]], axis=1).astype(bf)
        wv_h = np.ascontiguousarray(w_qkv[:, v0:v0 + HEAD_DIM]).astype(bf)
        in_maps.append({
            "xT": xT_h, "wqk": wqk_h, "wv": wv_h, "wout": wout_h,
            "cos2": cos_h, "sinS": sin_h,
        })

    res = run_bass_kernel_spmd(nc, in_maps, core_ids=list(range(NC)), trace=trace)
    full = np.concatenate([res.results[c]["out"] for c in range(NC)], axis=0)
    return full.reshape(B, S, HIDDEN), res.exec_time_ns


if __name__ == "__main__":
    rng = np.random.default_rng(0)
    x = rng.standard_normal((B, S, HIDDEN), dtype=np.float32)
    w_qkv = rng.standard_normal((HIDDEN, 6144), dtype=np.float32) * HIDDEN ** -0.5
    w_out = rng.standard_normal((HIDDEN, HIDDEN), dtype=np.float32) * HIDDEN ** -0.5
    o = kernel(x, w_qkv, w_out)
    print(o.shape, o.dtype)
